# revision 28
# baseline (speedup 1.0000x reference)
"""Trainium2 Bass kernel: nn_MultiHeadCrossAttention (B=4, S=1024, H=1024, 16 heads).

Sharding: 8 cores = (batch b in 0..3) x (head-group g in 0..1, 8 heads each).
Per core: q/k/v projections for its head group on its batch, flash-style
attention in scores-transposed layout (softmax along the PSUM partition axis
via an augmented ones-column in the v matmul), and a partial out-projection.
Host sums the two per-batch partials (fp64) and adds the output bias.

The bilinear span bias of the reference is constant along the softmax key
axis, so it cancels exactly in softmax and is not computed.

All matmul operands are fp16 (1 cycle/row on the PE), fp32 PSUM accumulate.

v2 structure (flattened pipeline): the 8 attention units (nq, head-pair) run
as one 64-step software pipeline with scores emitted 2 kt-steps ahead of the
ctx matmuls, crossing unit boundaries, so the PE never drains at a unit seam
(a drained PE also drops to a lower p-state: 0.65/1.2/2.4 GHz ramp).
The v/k/q projection chunks and the out-projection s-tiles are interleaved
into the pipeline as PE filler; out-proj st2-3 (which only depend on the nq0
half) are reserved for the tail to hide the last unit's normalize latency.

Softmax normalization: 64x-replicated ones-columns in the augmented v make
the ctx matmuls deposit per-query exp-sums on half the PSUM partitions.
Units 0-6 use the proven DMA-repartition + DRAM-bounce reciprocal broadcast
(latency-tolerant mid-kernel, all on the gpsimd queue). The LAST unit uses a
partition-shifted Ln + Exp(-x) on the Scalar engine instead (1/s with no DMA
hops); both ln and exp live in the natural_log_exp_and_others table set, so
there is at most one mid-kernel ACT_TABLE_LOAD.

Inputs are host-repartitioned to [128, ...] partition-major layouts so each
input is 1-2 large fully-contiguous DMAs, spread across 4 engine queues.
Output partials are fp16 (host sums in fp64; adds ~5e-4 rel err, gate 2e-2).
"""
import os
import sys
import types

sys.path.insert(0, "/opt/trn_rl_repo")

# Optional NTFF profile hook shim (axon images lack antenv.axon_hooks).
if "antenv.axon_hooks" not in sys.modules:
    try:
        import trn_agent_boot.trn_boot as _tb

        _m = types.ModuleType("antenv.axon_hooks")
        _m.get_axon_ntff_profile_hook = (
            lambda: _tb._ntff_profile_via_ctypes("/opt/axon/libaxon_pjrt.so")
        )
        _m.set_axon_ntff_profile_hook = lambda h: None
        sys.modules["antenv.axon_hooks"] = _m
    except Exception:
        pass

import numpy as np

import concourse.bass as bass
import concourse.mybir as mybir
import concourse.tile as tile
from concourse import bacc
from concourse.bass_utils import run_bass_kernel_spmd

F32 = mybir.dt.float32
F16 = mybir.dt.float16
AF = mybir.ActivationFunctionType

B, S, H = 4, 1024, 1024
NHEADS = 16
HD = 64
G = 2                  # head groups (cores per batch)
NH = NHEADS // G       # 8 heads per core
NH2 = NH // 2          # 4 head pairs per core
F = NH * HD            # 512 per-core qkv features
HC = H // 128          # 8 contraction chunks for projections
KT = S // 128          # 8 key tiles
ST = S // 128          # 8 seq tiles
FC = F // 128          # 4 feature chunks
NQ = S // 512          # 2 query halves
SCALE = float(HD) ** -0.5
VREG = 256             # augmented v region per head pair

# Last-unit normalize via partition-shifted Ln+Exp on Scalar (vs DMA chain)
USE_LNEXP_TAIL = True

_CACHE: dict = {}


def _spread(thunks, n=8):
    """Place len(thunks) items evenly across n slots (None elsewhere)."""
    slots = [None] * n
    k = len(thunks)
    for i, t in enumerate(thunks):
        slots[i * n // k] = t
    return slots


def _build_nc():
    nc = bacc.Bacc("TRN2", target_bir_lowering=False, debug=False)

    # Host-repartitioned inputs: partition-major, fully contiguous rows.
    # wk/wq are fc-major so the fc0 slices (all scores(u0) needs) are small
    # early DMAs: a consumer waits on its queue's FIFO completion counter,
    # so every byte queued ahead of a dependency delays it.
    xtp = nc.dram_tensor("xtp", [128, NQ, HC, 512], F16, kind="ExternalInput")
    ytp = nc.dram_tensor("ytp", [128, HC, S], F16, kind="ExternalInput")
    wqp = nc.dram_tensor("wqp", [FC, 128, HC, 128], F16, kind="ExternalInput")
    wkp = nc.dram_tensor("wkp", [FC, 128, HC, 128], F16, kind="ExternalInput")
    wvp = nc.dram_tensor("wvp", [128, HC, F], F16, kind="ExternalInput")
    wop = nc.dram_tensor("wop", [128, FC, H], F16, kind="ExternalInput")
    bqv = nc.dram_tensor("bqv", [F], F32, kind="ExternalInput")
    bkv = nc.dram_tensor("bkv", [F], F32, kind="ExternalInput")
    ebias = nc.dram_tensor("ebias", [S], F32, kind="ExternalInput")
    out = nc.dram_tensor("out", [S, H], F16, kind="ExternalOutput")
    # DRAM bounce for the units-0..6 softmax reciprocals (DRAM APs allow the
    # 0-stride partition-broadcast read that SBUF APs reject).
    rsc = nc.dram_tensor("rsc", [8, 2, 512], F32)

    UNITS = [(nq, hp) for nq in range(NQ) for hp in range(NH2)]

    with tile.TileContext(nc) as tc:
        const = tc.alloc_tile_pool(name="const", bufs=1)
        persist = tc.alloc_tile_pool(name="persist", bufs=1)

        bq_sb = const.tile([128, FC], F32, name="bq_sb")
        bk_sb = const.tile([128, FC], F32, name="bk_sb")
        eb_sb = const.tile([128, KT], F32, name="eb_sb")
        wo_sb = const.tile([128, FC, H], F16, name="wo_sb")
        wv_sb = const.tile([128, HC, F], F16, name="wv_sb")
        xt_sb = const.tile([128, NQ, HC, 512], F16, name="xt_sb")
        # per-chunk tiles: consumers go chunk-by-chunk behind the DMA queue
        wq_sbs = [const.tile([128, HC, 128], F16, name=f"wq_sb{fc}")
                  for fc in range(FC)]
        wk_sbs = [const.tile([128, HC, 128], F16, name=f"wk_sb{fc}")
                  for fc in range(FC)]
        yt_sbs = [const.tile([128, S], F16, name=f"yt_sb{hc}")
                  for hc in range(HC)]
        # ones/zeros stationary rows for the PE-broadcast normalize
        ones_t = const.tile([1, 256], F16, name="ones_t")

        qT_sb = persist.tile([128, FC, S], F16, name="qT_sb")
        kT_fcs = [persist.tile([128, S], F16, name=f"kT_fc{fc}")
                  for fc in range(FC)]
        # v per key-tile: ctx(u0, kt) must only depend on its own v chunk so
        # the v-projection can run as in-pipeline filler for unit 0.
        v_sbs = [persist.tile([128, NH2, VREG], F16, name=f"v_sb{kt}")
                 for kt in range(KT)]
        ctx_sbs = [persist.tile([128, FC, 512], F16, name=f"ctx_sb{nq}")
                   for nq in range(NQ)]

        # ---------------- input loads: 3 queues, need-ordered ----------
        # k0 streams per-chunk behind the yt/wk DMAs; q00 needs only the
        # fc0 slice of wq plus xt0; wo/xt1 are late-needed bulk at the back.
        nc.sync.dma_start(out=wk_sbs[0], in_=wkp[0])
        nc.gpsimd.dma_start(out=bk_sb, in_=bkv.rearrange("(c p) -> p c", p=128))
        for hc in range(4):
            nc.scalar.dma_start(out=yt_sbs[hc], in_=ytp[:, hc, :])
            nc.gpsimd.dma_start(out=yt_sbs[hc + 4], in_=ytp[:, hc + 4, :])
        nc.sync.dma_start(out=xt_sb[:, 0], in_=xtp[:, 0])
        nc.scalar.dma_start(out=wq_sbs[0], in_=wqp[0])
        nc.sync.dma_start(out=bq_sb, in_=bqv.rearrange("(c p) -> p c", p=128))
        nc.scalar.dma_start(out=eb_sb, in_=ebias.rearrange("(c p) -> p c", p=128))
        nc.gpsimd.dma_start(out=wv_sb, in_=wvp[:, :, :])
        for fc in range(1, FC):
            nc.sync.dma_start(out=wk_sbs[fc], in_=wkp[fc])
            nc.scalar.dma_start(out=wq_sbs[fc], in_=wqp[fc])
        nc.gpsimd.dma_start(out=xt_sb[:, 1], in_=xtp[:, 1])
        nc.sync.dma_start(out=wo_sb, in_=wop[:, :, :])

        # ones columns of the augmented v regions (DVE is idle at start)
        for kt in range(KT):
            nc.vector.memset(v_sbs[kt][:, :, 64:192], 1.0)
        # stationary rows for the PE-broadcast normalize: [1,256] with
        # cols 0:64 = 1 (e-half) and 192:256 = 1 (o-half), rest 0.
        nc.vector.memset(ones_t, 0.0)
        nc.vector.memset(ones_t[:, 0:64], 1.0)
        nc.vector.memset(ones_t[:, 192:256], 1.0)

        psum = tc.alloc_tile_pool(name="psum", bufs=1, space="PSUM")
        exps = tc.alloc_tile_pool(name="exps", bufs=4)
        outsb = tc.alloc_tile_pool(name="outsb", bufs=3)
        smallp = tc.alloc_tile_pool(name="smallp", bufs=2)
        ctxup = tc.alloc_tile_pool(name="ctxup", bufs=3)

        # ---------------- building blocks ----------------
        def proj_chunk(src_ap, w_sbs, b_sb, dst_ap, fc, nq):
            ps = psum.tile([128, 512], F32, name="qkps", tag="pp", bufs=2)
            for hc in range(HC):
                nc.tensor.matmul(
                    ps,
                    w_sbs[fc][:, hc, :],
                    src_ap(hc, nq),
                    start=(hc == 0), stop=(hc == HC - 1),
                )
            nc.vector.tensor_scalar_add(dst_ap, ps, b_sb[:, fc:fc + 1])

        yt_ap = lambda hc, nq: yt_sbs[hc][:, nq * 512:(nq + 1) * 512]
        xt_ap = lambda hc, nq: xt_sb[:, nq, hc, :]

        def k_half(fc, nq):
            proj_chunk(yt_ap, wk_sbs, bk_sb,
                       kT_fcs[fc][:, nq * 512:(nq + 1) * 512], fc, nq)

        def q_chunk(fc, nq):
            proj_chunk(xt_ap, wq_sbs, bq_sb,
                       qT_sb[:, fc, nq * 512:(nq + 1) * 512], fc, nq)

        def v_st(st):
            ps = psum.tile([128, F], F32, name="vps", tag="pp", bufs=2)
            for hc in range(HC):
                nc.tensor.matmul(
                    ps,
                    yt_sbs[hc][:, st * 128:(st + 1) * 128],
                    wv_sb[:, hc, :],
                    start=(hc == 0), stop=(hc == HC - 1),
                )
            pv = ps.rearrange("p (hp e d) -> p hp e d", hp=NH2, e=2)
            nc.vector.tensor_copy(v_sbs[st][:, :, 0:64], pv[:, :, 0, :])
            nc.vector.tensor_copy(v_sbs[st][:, :, 192:256], pv[:, :, 1, :])

        def outproj(st, no, scalar_ot=False):
            ps = psum.tile([128, 512], F32, name="ops", tag="pp", bufs=2)
            for fc2 in range(FC):
                nc.tensor.matmul(
                    ps,
                    ctx_sbs[st // 4][:, fc2,
                                     (st % 4) * 128:(st % 4 + 1) * 128],
                    wo_sb[:, fc2, no * 512:(no + 1) * 512],
                    start=(fc2 == 0), stop=(fc2 == FC - 1),
                )
            ot = outsb.tile([128, 512], F16, name="ot", tag="ot")
            if scalar_ot:
                # post-exp tail: Scalar is idle after the last exp, and the
                # in-order DVE queue must stay clear for the u6/u7 normalize
                # (ctxu copies + muls) that gates st4-7.
                nc.scalar.copy(ot, ps)
            else:
                nc.vector.tensor_copy(ot, ps)
            nc.sync.dma_start(
                out=out[st * 128:(st + 1) * 128, no * 512:(no + 1) * 512],
                in_=ot)

        ex_pending = {}

        def scores_exp(s):
            u, kt = divmod(s, 8)
            nq, hp = UNITS[u]
            fc = hp
            sps = psum.tile([128, 2, 512], F32, name="sps", tag="sps", bufs=2)
            for e in range(2):
                p0 = 64 * e
                nc.tensor.matmul(
                    sps[:, e, :],
                    kT_fcs[fc][p0:p0 + 64, kt * 128:(kt + 1) * 128],
                    qT_sb[p0:p0 + 64, fc, nq * 512:(nq + 1) * 512],
                    start=True, stop=True,
                )
            ex = exps.tile([128, 2, 512], F16, name="ex", tag="ex", bufs=6)
            nc.scalar.activation(
                ex, sps, AF.Exp, bias=eb_sb[:, kt:kt + 1], scale=SCALE)
            ex_pending[s] = ex

        def normalize(u, cps_e, cps_o):
            nq, hp = UNITS[u]
            fc = hp
            # full PSUM tiles (ctx + replicated sums rows) to fp16 SBUF --
            # frees the accumulator banks for the next unit.
            ctxu = ctxup.tile([128, 2, 512], F16, name="ctxu", tag="ctxu")
            nc.vector.tensor_copy(ctxu[:, 0, :], cps_e)
            nc.vector.tensor_copy(ctxu[:, 1, :], cps_o)
            sp = smallp.tile([128, 8], F16, name="sp", tag="sp")
            dq = nc.gpsimd
            dq.dma_start(out=sp[:, 0:4], in_=ctxu[64:65, 0, :])
            dq.dma_start(out=sp[:, 4:8], in_=ctxu[0:1, 1, :])
            # sp[p, e*4+c] = sums_{e}[4p+c]
            if u >= len(UNITS) - 2:
                # tail units gate out-proj st4-7: broadcast 1/sums across
                # partitions on the PE (one repartition DMA + two 1-row
                # stationary matmuls into PSUM) -- far lower latency than
                # the DRAM 0-stride bounce. The PE matmuls + muls are
                # DEFERRED to the tail so the in-order PE queue never
                # stalls on this chain at a unit seam.
                rp = smallp.tile([128, 8], F16, name="rp16", tag="rp16")
                with nc.allow_low_precision(
                        reason="fp16 1/sums matches the fp16 sums rows"):
                    nc.vector.reciprocal(out=rp, in_=sp)
                rrow = smallp.tile([1, 1024], F16, name="rrow", tag="rrow")
                # rrow[0, e*512 + 4p+c] = rp[p, e*4+c] = 1/sums_e[4p+c]
                dq.dma_start(out=rrow[:, 0:512], in_=rp[:, 0:4])
                dq.dma_start(out=rrow[:, 512:1024], in_=rp[:, 4:8])

                def phase_b():
                    rt = psum.tile([128, 512], F32, name="rtps", tag="pp",
                                   bufs=2)
                    nc.tensor.matmul(rt, ones_t[:, 0:128], rrow[:, 0:512],
                                     start=True, stop=False)
                    nc.tensor.matmul(rt, ones_t[:, 128:256],
                                     rrow[:, 512:1024],
                                     start=False, stop=True)
                    nc.vector.tensor_mul(
                        ctx_sbs[nq][0:64, fc, :], ctxu[0:64, 0, :],
                        rt[0:64, :])
                    nc.vector.tensor_mul(
                        ctx_sbs[nq][64:128, fc, :], ctxu[64:128, 1, :],
                        rt[64:128, :])
                return phase_b
            # mid-kernel: latency-tolerant DRAM bounce of the exact DVE
            # reciprocal, broadcast-read back with a 0-stride partition
            # AP (single merged DMA each way).
            rp = smallp.tile([128, 8], F32, name="rp", tag="rp")
            nc.vector.reciprocal(out=rp, in_=sp)
            dq.dma_start(out=rsc[u, 0, :], in_=rp[:, 0:4])
            dq.dma_start(out=rsc[u, 1, :], in_=rp[:, 4:8])
            rt = smallp.tile([128, 512], F32, name="rt", tag="rt")
            for e in range(2):
                src = rsc[u, e, :]
                dq.dma_start(
                    out=rt[64 * e:64 * e + 64, :],
                    in_=bass.AP(tensor=src.tensor, offset=src.offset,
                                ap=[[0, 64]] + list(src.ap)))
            nc.gpsimd.tensor_mul(
                ctx_sbs[nq][0:64, fc, :], ctxu[0:64, 0, :], rt[0:64, :])
            nc.gpsimd.tensor_mul(
                ctx_sbs[nq][64:128, fc, :], ctxu[64:128, 1, :],
                rt[64:128, :])
            return None

        # ---------------- filler schedule ----------------
        def vpair(j):
            def f():
                v_st(2 * j)
                v_st(2 * j + 1)
            return f

        def kh(fc, nq):
            return lambda: k_half(fc, nq)

        def qc(fc, nq):
            return lambda: q_chunk(fc, nq)

        def op(st, no):
            return lambda: outproj(st, no)

        fillers = {
            0: [vpair(0), vpair(1), vpair(2), vpair(3),
                kh(1, 0), kh(1, 1), qc(1, 0), None],
            1: _spread([kh(2, 0), qc(2, 0), kh(2, 1)]),
            2: _spread([kh(3, 0), qc(3, 0), kh(3, 1)]),
            3: _spread([qc(0, 1)]),
            4: _spread([qc(1, 1), op(0, 0)]),
            5: _spread([qc(2, 1), op(0, 1), op(1, 0)]),
            6: _spread([qc(3, 1), op(1, 1), op(2, 0)]),
            7: _spread([op(2, 1)]),
        }

        # ---------------- the 64-step pipeline ----------------
        k_half(0, 0)
        k_half(0, 1)
        q_chunk(0, 0)
        scores_exp(0)
        scores_exp(1)

        cur = {}
        deferred = []
        for s in range(64):
            u, kt = divmod(s, 8)
            if kt == 0:
                cur["e"] = psum.tile([128, 512], F32, name="cps_e",
                                     tag="cpse", bufs=1)
                cur["o"] = psum.tile([128, 512], F32, name="cps_o",
                                     tag="cpso", bufs=1)
            th = fillers[u][kt]
            if th is not None:
                th()
            # ctx BEFORE scores(s+2): scores' stationary kT/qT come from DVE
            # bias-adds of just-emitted filler chunks; ctx(s) (whose ex is
            # long ready) gives the add time to land without a PE bubble.
            ex = ex_pending.pop(s)
            nq, hp = UNITS[u]
            nc.tensor.matmul(
                cur["e"], v_sbs[kt][:, hp, 0:128], ex[:, 0, :],
                start=(kt == 0), stop=(kt == KT - 1))
            nc.tensor.matmul(
                cur["o"], v_sbs[kt][:, hp, 128:256], ex[:, 1, :],
                start=(kt == 0), stop=(kt == KT - 1))
            if s + 2 < 64:
                scores_exp(s + 2)
            if kt == 7:
                pb = normalize(u, cur["e"], cur["o"])
                if pb is not None:
                    deferred.append(pb)

        # tail: st2b/st3 (nq0-only deps) cover the deferred normalize
        # chains' DMA latency; then the PE-broadcast normalizes of u6/u7
        # complete ctx_sbs[1], unblocking st4-7.
        deferred[0]()          # u6 phase B (its rrow landed long ago)
        outproj(3, 0, scalar_ot=True)
        outproj(3, 1, scalar_ot=True)
        deferred[1]()          # u7 phase B
        for st in range(4, 8):
            for no in range(NQ):
                outproj(st, no, scalar_ot=True)

        ctxup.release()
        smallp.release()
        outsb.release()
        exps.release()
        psum.release()
        persist.release()
        const.release()

    nc.finalize()
    return nc


def get_nc():
    if "nc" not in _CACHE:
        _CACHE["nc"] = _build_nc()
    return _CACHE["nc"]


def make_in_maps(aspect_hidden, opinion_hidden, attention_mask,
                 Wq, bq, Wk, bk, Wv, bv, Wo, bo):
    asp = np.asarray(aspect_hidden, np.float32)
    opi = np.asarray(opinion_hidden, np.float32)
    mask = np.asarray(attention_mask)

    def shuffle_w(wT):  # [H, F] -> [128, HC, F]
        return np.ascontiguousarray(
            wT.reshape(HC, 128, F).transpose(1, 0, 2).astype(np.float16))

    def shuffle_w_fc(wT):  # [H, F] -> [FC, 128, HC, 128] (fc-major chunks)
        return np.ascontiguousarray(
            wT.reshape(HC, 128, FC, 128).transpose(2, 1, 0, 3)
            .astype(np.float16))

    def shuffle_x(xT):  # [H, S] -> [128, NQ, HC, 512]
        return np.ascontiguousarray(
            xT.reshape(HC, 128, NQ, 512).transpose(1, 2, 0, 3)
            .astype(np.float16))

    def shuffle_y(yT):  # [H, S] -> [128, HC, S]
        return np.ascontiguousarray(
            yT.reshape(HC, 128, S).transpose(1, 0, 2).astype(np.float16))

    xts = [shuffle_x(asp[b].T) for b in range(B)]
    yts = [shuffle_y(opi[b].T) for b in range(B)]
    ebs = [np.where(mask[b] == 0, np.float32(-1e30),
                    np.float32(0.0)).astype(np.float32) for b in range(B)]
    wqs = [shuffle_w_fc(Wq[g * F:(g + 1) * F, :].T) for g in range(G)]
    wks = [shuffle_w_fc(Wk[g * F:(g + 1) * F, :].T) for g in range(G)]
    wvs = [shuffle_w(Wv[g * F:(g + 1) * F, :].T) for g in range(G)]
    wos = [np.ascontiguousarray(
        Wo[:, g * F:(g + 1) * F].T.reshape(FC, 128, H).transpose(1, 0, 2)
        .astype(np.float16)) for g in range(G)]
    bqs = [np.ascontiguousarray(bq[g * F:(g + 1) * F]) for g in range(G)]
    bks = [np.ascontiguousarray(bk[g * F:(g + 1) * F]) for g in range(G)]
    in_maps = []
    for c in range(8):
        b, g = c // G, c % G
        in_maps.append({
            "xtp": xts[b], "ytp": yts[b],
            "wqp": wqs[g], "wkp": wks[g], "wvp": wvs[g], "wop": wos[g],
            "bqv": bqs[g], "bkv": bks[g], "ebias": ebs[b],
        })
    return in_maps


def kernel(aspect_hidden, opinion_hidden, attention_mask,
           Wq, bq, Wk, bk, Wv, bv, Wo, bo, Wbil, bbil):
    Wq = np.asarray(Wq, np.float32); bq = np.asarray(bq, np.float32)
    Wk = np.asarray(Wk, np.float32); bk = np.asarray(bk, np.float32)
    Wv = np.asarray(Wv, np.float32); bv = np.asarray(bv, np.float32)
    Wo = np.asarray(Wo, np.float32); bo = np.asarray(bo, np.float32)

    nc = get_nc()
    in_maps = make_in_maps(aspect_hidden, opinion_hidden, attention_mask,
                           Wq, bq, Wk, bk, Wv, bv, Wo, bo)
    trace = bool(int(os.environ.get("KERNEL_TRACE", "0")))
    res = run_bass_kernel_spmd(nc, in_maps, core_ids=list(range(8)),
                               trace=trace)
    _CACHE["last_results"] = res

    # v-bias folds into a constant output offset: softmax rows sum to 1, so
    # ctx picks up +bv exactly, and out picks up +Wo @ bv.
    bo_eff = (bo.astype(np.float64)
              + Wo.astype(np.float64) @ bv.astype(np.float64))
    outs = np.empty((B, S, H), np.float32)
    for b in range(B):
        acc = (res.results[G * b]["out"].astype(np.float64)
               + res.results[G * b + 1]["out"].astype(np.float64) + bo_eff)
        outs[b] = acc.astype(np.float32)
    return outs
